# revision 23
# baseline (speedup 1.0000x reference)
"""CLIP attention (B=4, S=2048, E=1024, H=16, D=64) on 8 Trainium2 cores.

Sharding: core c handles batch b = c // 2 and heads [ (c%2)*8, (c%2)*8+8 ).
Each core computes its 8 heads' attention plus its partial output
projection (contraction over its 512 local context dims); the host sums
the two partials per batch and adds the output bias.

Per-core dataflow (all activations stored transposed, [feature, seq]):
  hT [E, S]            <- host-pretransposed hidden_states[b], bf16
  qT, kT [512, S]      =  Wq_loc @ hT (+bias, query pre-scaled)   on PE
  v    [S, 512]        =  hT.T @ Wv_loc.T (+bias via bcast add), stored
                          as v_ext tiles [128, 8*65] with a ones column
                          per head (fused softmax denominator)
  ST   [k, q]          =  kT_h.T-slices @ qT_h  (scores, transposed;
                          two heads packed in PE row groups 0-63/64-127)
  P^T  = exp(ST)       on ACT, PSUM -> SBUF bf16 (no max subtraction:
                          scores ~ N(0,1), exp is safe in fp32)
  outT_ext [65, q]     =  V_ext.T @ P^T accumulated over k tiles; row 64
                          is the softmax denominator (ones column)
  CT   [512, S]        =  outT * (1/denom) broadcast  (DVE reciprocal,
                          PE ones-matmul partition broadcast, DVE mul)
  outT_partial [E, S]  =  Wo_loc^T-slices @ CT  -> DRAM fp32

Scheduling (measured on HW: PE matmul work ~300us/core, ACT exp ~246us,
total bounded by the PE stream + per-instruction sync overhead):
  - PSUM banks: 4 = two [128,1024] score tiles (score/exp pipeline),
    2 = per-head [65,512] AV accumulators, 2 = projection-chain /
    out-projection / 1/denom-broadcast tiles.
  - AV matmuls run 3 k-tiles behind scores/exp, so the sweep-boundary
    normalize (which frees the previous sweep's av slots on DVE) never
    stalls the PE.
  - q/k projection chains for head-pair p+1 are paced 2 per sweep
    (deadline-ordered: q0,k0,k1..k3,q1..q3) into pair p's sweeps;
    finished s-chunks' output projections fill the remaining slack.
"""

import numpy as np

B, S, E = 4, 2048, 1024
H, D = 16, 64
SCALE = D ** -0.5
NCORES = 8
HLOC = 8            # heads per core
CLOC = HLOC * D     # 512 local context dims
NHP = HLOC // 2     # 4 head pairs
SC = 512            # seq chunk (matmul moving free dim)
NQC = S // SC       # 4
KT = 128            # k tile rows
NKT = S // KT       # 16
NE = E // 128       # 8 contraction chunks for projections
VW = D + 1          # 65: v columns + fused ones column

_CACHE = {}


def _get_deps():
    import sys
    if "/opt/trn_rl_repo" not in sys.path:
        sys.path.insert(0, "/opt/trn_rl_repo")
    import concourse.bass as bass
    import concourse.mybir as mybir
    import concourse.tile as tile
    return bass, mybir, tile


def _fix_multi_waits(nc, mybir):
    """walrus encodes at most ONE semaphore wait per TPB engine
    instruction. Move surplus waits onto a same-engine Drain inserted just
    before the offending instruction (Drains accept many waits)."""
    for f in nc.m.functions:
        for bb in f.blocks:
            ins = bb.instructions
            if not any(i.sync_info and len(i.sync_info.on_wait) > 1
                       for i in ins):
                continue
            out = []
            for i in ins:
                if i.sync_info and len(i.sync_info.on_wait) > 1:
                    w = list(i.sync_info.on_wait)
                    # a wait on the instruction's OWN processor semaphore
                    # is implied by that processor's FIFO order - drop it
                    own = {u.ant_name for u in i.sync_info.on_update}
                    w2 = [x for x in w if x.ant_name not in own]
                    if not w2:
                        w2 = w[-1:]
                    for j, wj in enumerate(w2[:-1]):
                        d = mybir.InstDrain(
                            name=f"{i.name}_wj{j}", ins=[], outs=[],
                            bass_is_fusable=False)
                        d.engine = i.engine
                        d.sync_info = mybir.SyncInfo(on_wait=[wj], on_update=[])
                        out.append(d)
                    i.sync_info = mybir.SyncInfo(
                        on_wait=w2[-1:], on_update=list(i.sync_info.on_update))
                out.append(i)
            bb.instructions = out


def build_program(fix_waits=True, reps=1, probe=None):
    """Build the single-core Bass/Tile program (same program on all cores).

    fix_waits: apply the walrus 1-wait-per-instruction fixup (required for
    hardware NEFF compile; CoreSim's race detector can't digest the
    inserted drains, so sim runs pass False).
    probe: None for the real kernel; "noact" replaces the softmax exp with
    a constant tile (AV reads it) to time the pure PE stream."""
    bass, mybir, tile = _get_deps()
    from contextlib import ExitStack

    f32 = mybir.dt.float32
    bf16 = mybir.dt.bfloat16
    EXP = mybir.ActivationFunctionType.Exp

    nc = bass.Bass()

    hT_d = nc.declare_dram_parameter("hT", [E, S], bf16, isOutput=False)
    wqT_d = nc.declare_dram_parameter("wqT", [E, CLOC], bf16, isOutput=False)
    wkT_d = nc.declare_dram_parameter("wkT", [E, CLOC], bf16, isOutput=False)
    wvT_d = nc.declare_dram_parameter("wvT", [E, CLOC], bf16, isOutput=False)
    woT_d = nc.declare_dram_parameter("woT", [CLOC, E], bf16, isOutput=False)
    bq_d = nc.declare_dram_parameter("bq", [CLOC], f32, isOutput=False)
    bk_d = nc.declare_dram_parameter("bk", [CLOC], f32, isOutput=False)
    bv_d = nc.declare_dram_parameter("bv", [CLOC], f32, isOutput=False)
    outT_d = nc.declare_dram_parameter("outT", [E, S], f32, isOutput=True)

    add = mybir.AluOpType.add
    mult = mybir.AluOpType.mult

    with tile.TileContext(nc) as tc, ExitStack() as ctx:
        sb = ctx.enter_context(tc.tile_pool(name="persist", bufs=1))

        # ---- persistent SBUF tiles ----
        h_sb = [sb.tile([128, S], bf16, name=f"h{e}", tag=f"h{e}") for e in range(NE)]
        wq_sb = [sb.tile([128, CLOC], bf16, name=f"wq{e}", tag=f"wq{e}") for e in range(NE)]
        wk_sb = [sb.tile([128, CLOC], bf16, name=f"wk{e}", tag=f"wk{e}") for e in range(NE)]
        wv_sb = [sb.tile([128, CLOC], bf16, name=f"wv{e}", tag=f"wv{e}") for e in range(NE)]
        wo_sb = [sb.tile([128, E], bf16, name=f"wo{c}", tag=f"wo{c}") for c in range(4)]
        qT_sb = [sb.tile([128, S], bf16, name=f"qT{p}", tag=f"qT{p}") for p in range(NHP)]
        kT_sb = [sb.tile([128, S], bf16, name=f"kT{p}", tag=f"kT{p}") for p in range(NHP)]
        vx_sb = [sb.tile([128, HLOC * VW], bf16, name=f"vx{t}", tag=f"vx{t}") for t in range(NKT)]
        ct_sb = [sb.tile([128, S], bf16, name=f"ct{p}", tag=f"ct{p}") for p in range(NHP)]
        bq_sb = sb.tile([128, 4], f32, name="bq_sb", tag="bq_sb")
        bk_sb = sb.tile([128, 4], f32, name="bk_sb", tag="bk_sb")
        bv_sb = sb.tile([1, CLOC], f32, name="bv_sb", tag="bv_sb")
        bvb_sb = sb.tile([128, CLOC], f32, name="bvb_sb", tag="bvb_sb")
        ones1 = sb.tile([1, 128], f32, name="ones1", tag="ones1")
        ones64 = sb.tile([1, 64], bf16, name="ones64", tag="ones64")

        # ---- input DMAs, ordered by first use, 128 KB chunks so the
        # 8 hardware DMA queues stream them in parallel ----
        nc.sync.dma_start(out=bv_sb[:], in_=bv_d[:])
        for dtile in range(4):
            r = slice(dtile * 128, (dtile + 1) * 128)
            nc.sync.dma_start(out=bq_sb[:, dtile:dtile + 1], in_=bq_d[r])
            nc.sync.dma_start(out=bk_sb[:, dtile:dtile + 1], in_=bk_d[r])
        for e in range(NE):
            r = slice(e * 128, (e + 1) * 128)
            nc.sync.dma_start(out=wv_sb[e][:], in_=wvT_d[r, :])
        scol0 = slice(0, SC)
        for e in range(NE):
            r = slice(e * 128, (e + 1) * 128)
            nc.sync.dma_start(out=h_sb[e][:, scol0], in_=hT_d[r, scol0])
        for e in range(NE):
            r = slice(e * 128, (e + 1) * 128)
            nc.sync.dma_start(out=wq_sb[e][:], in_=wqT_d[r, :])
        for e in range(NE):
            r = slice(e * 128, (e + 1) * 128)
            nc.sync.dma_start(out=wk_sb[e][:], in_=wkT_d[r, :])
        for sc in range(1, NQC):
            scol = slice(sc * SC, (sc + 1) * SC)
            for e in range(NE):
                r = slice(e * 128, (e + 1) * 128)
                nc.sync.dma_start(out=h_sb[e][:, scol], in_=hT_d[r, scol])
        for c in range(4):
            nc.sync.dma_start(out=wo_sb[c][:], in_=woT_d[c * 128:(c + 1) * 128, :])
        nc.vector.memset(ones1[:], 1.0)
        nc.vector.memset(ones64[:], 1.0)
        ones_ex = None
        if probe == "noact":
            ones_ex = sb.tile([128, 2 * SC], bf16, name="ones_ex",
                              tag="ones_ex")
            nc.vector.memset(ones_ex[:], 0.001)

        # ones columns of v_ext (softmax denominator fusion), set once
        for t in range(NKT):
            for h in range(HLOC):
                nc.vector.memset(vx_sb[t][:, h * VW + D:h * VW + D + 1], 1.0)

        for _rep in range(reps):
            with tc.tile_pool(name="ppj", bufs=2, space="PSUM") as ppj, \
                 tc.tile_pool(name="avp", bufs=2, space="PSUM") as avp, \
                 tc.tile_pool(name="stp", bufs=2, space="PSUM") as stp, \
                 tc.tile_pool(name="exs", bufs=18) as exs, \
                 tc.tile_pool(name="nrm", bufs=4) as nrm, \
                 tc.tile_pool(name="ost", bufs=4) as ost:
                # PSUM budget (8 banks of 2KB):
                #   stp "st"  [128,1024] f32 x2 = 4 banks (score tiles)
                #   avp "av"  [VW, 512] f32 x2  = 2 banks (per-head AV
                #             accumulators)
                #   ppj "pps" [128, 512] f32 x2 = 2 banks (chains,
                #             out-projection, 1/denom broadcast)
                # AV matmuls run 3 k-tiles behind the score/exp pipeline so
                # the sweep-boundary normalize (which frees the previous
                # sweep's av slots) never stalls them.

                def qk_chain(w_sb, b_sb, dst_sb, p, scnk):
                    dcol = slice(p * 128, (p + 1) * 128)
                    scol = slice(scnk * SC, (scnk + 1) * SC)
                    ps = ppj.tile([128, SC], f32, name="qkps", tag="pps")
                    for e in range(NE):
                        nc.tensor.matmul(
                            ps[:], w_sb[e][:, dcol], h_sb[e][:, scol],
                            start=(e == 0), stop=(e == NE - 1),
                            skip_group_check=True)
                    nc.vector.tensor_scalar(
                        dst_sb[p][:, scol], ps[:], b_sb[:, p:p + 1],
                        None, op0=add)

                def v_chain(st):
                    ps = ppj.tile([128, CLOC], f32, name="vps", tag="pps")
                    for e in range(NE):
                        nc.tensor.matmul(
                            ps[:], h_sb[e][:, st * 128:(st + 1) * 128], wv_sb[e][:],
                            start=(e == 0), stop=(e == NE - 1))
                    nc.vector.tensor_tensor(
                        vx_sb[st][:].rearrange("p (h w) -> p h w", w=VW)[:, :, 0:D],
                        ps[:].rearrange("p (h w) -> p h w", w=D),
                        bvb_sb[:].rearrange("p (h w) -> p h w", w=D),
                        op=add)

                def out_proj_tile(scnk, et):
                    scol = slice(scnk * SC, (scnk + 1) * SC)
                    erow = slice(et * 128, (et + 1) * 128)
                    ps = ppj.tile([128, SC], f32, name="ops", tag="pps")
                    for c in range(4):
                        nc.tensor.matmul(
                            ps[:], wo_sb[c][:, erow], ct_sb[c][:, scol],
                            start=(c == 0), stop=(c == 3))
                    ot = ost.tile([128, SC], f32, name="ot", tag="ot")
                    nc.vector.tensor_copy(ot[:], ps[:])
                    nc.sync.dma_start(out=outT_d[erow, scol], in_=ot[:])

                # head: v-bias broadcast, the minimum projections the
                # first sweep needs (v tiles 0-3, q/k chunk 0 of pair 0);
                # everything else is interleaved into the sweeps
                bb_ps = ppj.tile([128, CLOC], f32, name="bbps", tag="pps")
                nc.tensor.matmul(bb_ps[:], ones1[:], bv_sb[:], start=True,
                                 stop=True)
                nc.vector.tensor_copy(bvb_sb[:], bb_ps[:])
                for st in range(4):
                    v_chain(st)
                qk_chain(wq_sb, bq_sb, qT_sb, 0, 0)
                qk_chain(wk_sb, bk_sb, kT_sb, 0, 0)

                # attention sweeps with PE filler work (q/k chains for the
                # next pair, out-projection for finished s-chunks) spread
                # into each sweep's slack
                chainq = []
                opq = []
                pending_norm = []

                def normalize(av, p, hh, qc):
                    # av: per-head [VW, SC] accumulator; row 64 = denom
                    qcol = slice(qc * SC, (qc + 1) * SC)
                    rr = nrm.tile([1, SC], bf16, name="rr", tag="rr")
                    with nc.allow_low_precision("1/denom in bf16: <=2^-9 rel"):
                        nc.vector.reciprocal(rr[:], av[64:65, :])
                    bc = ppj.tile([64, SC], f32, name="bc", tag="pps")
                    nc.tensor.matmul(bc[:], ones64[:], rr[0:1, :],
                                     start=True, stop=True)
                    rb = nrm.tile([64, SC], f32, name="rb", tag="rb")
                    nc.vector.tensor_copy(rb[:], bc[:])
                    nc.vector.tensor_tensor(
                        ct_sb[p][hh * 64:(hh + 1) * 64, qcol], av[0:64, :],
                        rb[:], op=mult)
                    if p == NHP - 1 and hh == 1:
                        for et in range(NE):
                            opq.append(
                                (lambda s=qc, e=et: out_proj_tile(s, e)))

                # first sweep's chain work, ordered by first use inside
                # the (p0, qc0) k-tile loop: k-chunks land before their
                # score tiles, v tiles before their AV matmuls
                for s in range(1, NQC):
                    chainq.append(
                        (lambda s=s: qk_chain(wk_sb, bk_sb, kT_sb, 0, s)))
                for st in range(4, NKT):
                    chainq.append((lambda st=st: v_chain(st)))
                # reorder: k1 first, then v4.., k2 after v6, k3 after v9
                chainq = [chainq[0], chainq[3], chainq[4], chainq[5],
                          chainq[1], chainq[6], chainq[7], chainq[8],
                          chainq[2]] + chainq[9:]
                for s in range(1, NQC):
                    chainq.append(
                        (lambda s=s: qk_chain(wq_sb, bq_sb, qT_sb, 0, s)))
                for p in range(NHP):
                    if p < NHP - 1:
                        # full chains for pair p+1, pulled 2 per sweep
                        # during pair p's sweeps; deadline order: q0/k0
                        # before the pair's first sweep, k1-k3 before that
                        # sweep's k-tiles 4/8/12, q1-q3 before sweeps 1-3
                        specs = ([(wq_sb, bq_sb, qT_sb, 0),
                                  (wk_sb, bk_sb, kT_sb, 0)] +
                                 [(wk_sb, bk_sb, kT_sb, s)
                                  for s in range(1, NQC)] +
                                 [(wq_sb, bq_sb, qT_sb, s)
                                  for s in range(1, NQC)])
                        for (w, b, d, s) in specs:
                            chainq.append(
                                (lambda w=w, b=b, d=d, s=s, q=p + 1:
                                 qk_chain(w, b, d, q, s)))
                    for qc in range(NQC):
                        qcol = slice(qc * SC, (qc + 1) * SC)
                        av = [avp.tile([VW, SC], f32, name=f"av{hh}",
                                       tag="av") for hh in range(2)]
                        exh = {}
                        dense = (p == 0 and qc == 0)
                        # k-tiles processed in batches: same-shape
                        # matmuls issue back-to-back (PE tile-config
                        # switches measure ~140ns each), scores+exp in
                        # batches of 2 (PSUM-limited), AV in batches of 4
                        # k-tiles running 5-8 k-tiles behind, which also
                        # keeps the sweep-boundary normalize off the
                        # critical path
                        for ktb in range(0, NKT + 2, 2):
                            for kt in (ktb, ktb + 1):
                                if kt >= NKT:
                                    break
                                kcol = slice(kt * 128, (kt + 1) * 128)
                                st_t = stp.tile([128, 2 * SC], f32, name="st",
                                                tag="st")
                                exh[kt] = st_t
                                nc.tensor.matmul(
                                    st_t[:, 0:SC], kT_sb[p][0:64, kcol],
                                    qT_sb[p][0:64, qcol],
                                    start=True, stop=True, tile_position=(0, 0))
                                nc.tensor.matmul(
                                    st_t[:, SC:2 * SC], kT_sb[p][64:128, kcol],
                                    qT_sb[p][64:128, qcol],
                                    start=True, stop=True, tile_position=(64, 0))
                            for kt in (ktb, ktb + 1):
                                if kt >= NKT:
                                    break
                                if probe == "noact":
                                    exh[kt] = ones_ex
                                else:
                                    ex = exs.tile([128, 2 * SC], bf16,
                                                  name="ex", tag="ex")
                                    nc.scalar.activation(ex[:], exh[kt][:],
                                                         EXP)
                                    exh[kt] = ex
                            # sweep-head normalize, emitted after the first
                            # score batch so the PE reaches the 1/denom
                            # broadcast matmuls only once the DVE
                            # reciprocals are done; frees the previous
                            # sweep's av slots before the AV matmuls need
                            # them
                            if ktb == 0:
                                while pending_norm:
                                    normalize(*pending_norm.pop(0))
                            if ktb == NKT:
                                for j in range(0, NKT):
                                    if j >= NKT:
                                        break
                                    pex = exh.pop(j)
                                    for hh in range(2):
                                        h = 2 * p + hh
                                        nc.tensor.matmul(
                                            av[hh][:],
                                            vx_sb[j][:, h * VW:h * VW + VW],
                                            pex[:, hh * SC:(hh + 1) * SC],
                                            start=(j == 0),
                                            stop=(j == NKT - 1),
                                            skip_group_check=True)
                            # filler work in the sweep's PE slack: the very
                            # first sweep pulls two chains per batch (it
                            # must produce the projections it consumes);
                            # later sweeps pace 2 chains per sweep and fill
                            # the rest with out-projection tiles
                            if dense and ktb < NKT and chainq:
                                chainq.pop(0)()
                                if chainq:
                                    chainq.pop(0)()
                            elif not dense and ktb in (2, 6) and chainq:
                                chainq.pop(0)()
                            elif not dense and ktb >= 4 and \
                                    ktb not in (2, 6) and opq:
                                opq.pop(0)()
                                if opq and ktb % 4 == 0:
                                    opq.pop(0)()
                        # normalize deferred into the next sweep's head
                        pending_norm.append((av[0], p, 0, qc))
                        pending_norm.append((av[1], p, 1, qc))
                while pending_norm:
                    normalize(*pending_norm.pop(0))
                # drain remaining out-projection tiles
                while opq:
                    opq.pop(0)()

    if fix_waits:
        _fix_multi_waits(nc, mybir)
    return nc


def make_inputs(hidden_states, Wq, bq, Wk, bk, Wv, bv, Wo, bo):
    """Shard + preprocess the full inputs into 8 per-core input maps.
    Shared pieces (per-batch hidden transpose, per-half weight slices)
    are computed once and referenced by both cores that use them."""
    import ml_dtypes
    bf16 = ml_dtypes.bfloat16
    f32 = np.float32

    hidden_states = np.asarray(hidden_states, f32)
    hT = [np.ascontiguousarray(hidden_states[b].T).astype(bf16)
          for b in range(B)]
    halves = []
    for half in range(2):
        hs = slice(half * CLOC, half * CLOC + CLOC)
        halves.append({
            "wqT": np.ascontiguousarray(
                (np.asarray(Wq, f32)[hs] * SCALE).T).astype(bf16),
            "wkT": np.ascontiguousarray(np.asarray(Wk, f32)[hs].T).astype(bf16),
            "wvT": np.ascontiguousarray(np.asarray(Wv, f32)[hs].T).astype(bf16),
            "woT": np.ascontiguousarray(np.asarray(Wo, f32)[:, hs].T).astype(bf16),
            "bq": np.ascontiguousarray(np.asarray(bq, f32)[hs] * SCALE),
            "bk": np.ascontiguousarray(np.asarray(bk, f32)[hs]),
            "bv": np.ascontiguousarray(np.asarray(bv, f32)[hs]),
        })
    return [{"hT": hT[c // 2], **halves[c % 2]} for c in range(NCORES)]


def gather_output(results, bo):
    out = np.empty((B, S, E), np.float32)
    bo = np.asarray(bo, np.float32)
    for b in range(B):
        acc = results[2 * b]["outT"].astype(np.float32) + \
              results[2 * b + 1]["outT"].astype(np.float32)
        out[b] = acc.T + bo
    return out


def _get_runner():
    """Build the Bass program + jitted 8-core executable once; reuse."""
    if "runner" in _CACHE:
        return _CACHE["runner"]
    _get_deps()
    import jax
    import numpy as np
    from jax.sharding import Mesh, PartitionSpec
    from jax.experimental.shard_map import shard_map
    from concourse import bass2jax, mybir

    bass2jax.install_neuronx_cc_hook()
    nc = build_program()

    partition_name = (nc.partition_id_tensor.name
                      if nc.partition_id_tensor else None)
    in_names, out_names, out_avals = [], [], []
    for alloc in nc.m.functions[0].allocations:
        if not isinstance(alloc, mybir.MemoryLocationSet):
            continue
        name = alloc.memorylocations[0].name
        if alloc.kind == "ExternalInput":
            if name != partition_name:
                in_names.append(name)
        elif alloc.kind == "ExternalOutput":
            out_names.append(name)
            out_avals.append(jax.core.ShapedArray(
                tuple(alloc.tensor_shape), mybir.dt.np(alloc.dtype)))
    n_params = len(in_names)
    all_in_names = in_names + out_names
    if partition_name is not None:
        all_in_names = all_in_names + [partition_name]

    def _body(*args):
        operands = list(args)
        if partition_name is not None:
            operands.append(bass2jax.partition_id_tensor())
        outs = bass2jax._bass_exec_p.bind(
            *operands,
            out_avals=tuple(out_avals),
            in_names=tuple(all_in_names),
            out_names=tuple(out_names),
            lowering_input_output_aliases=(),
            sim_require_finite=True,
            sim_require_nnan=True,
            nc=nc,
        )
        return tuple(outs)

    devices = jax.devices()[:NCORES]
    mesh = Mesh(np.asarray(devices), ("core",))
    n_outs = len(out_avals)
    sharded = jax.jit(
        shard_map(
            _body, mesh=mesh,
            in_specs=(PartitionSpec("core"),) * (n_params + n_outs),
            out_specs=(PartitionSpec("core"),) * n_outs,
            check_rep=False,
        ),
        donate_argnums=tuple(range(n_params, n_params + n_outs)),
        keep_unused=True,
    )

    def run(in_maps):
        concat_in = [
            np.concatenate([np.asarray(in_maps[c][nm]) for c in range(NCORES)],
                           axis=0)
            for nm in in_names
        ]
        concat_zeros = [
            np.zeros((NCORES * a.shape[0], *a.shape[1:]), a.dtype)
            for a in out_avals
        ]
        out_arrs = sharded(*concat_in, *concat_zeros)
        return [
            {nm: np.asarray(out_arrs[i]).reshape(NCORES, *out_avals[i].shape)[c]
             for i, nm in enumerate(out_names)}
            for c in range(NCORES)
        ]

    _CACHE["runner"] = (run, sharded, in_names, out_avals)
    return _CACHE["runner"]


def kernel(hidden_states, Wq, bq, Wk, bk, Wv, bv, Wo, bo):
    run = _get_runner()[0]
    in_maps = make_inputs(hidden_states, Wq, bq, Wk, bk, Wv, bv, Wo, bo)
    results = run(in_maps)
    return gather_output(results, bo)


def bench(in_maps, iters=20, pipeline=True):
    """Time repeated device executions with device-resident inputs and a
    non-donating jit (zeros reused). Returns per-iter seconds."""
    import time
    import numpy as np
    import jax
    from jax.sharding import Mesh, NamedSharding, PartitionSpec
    from jax.experimental.shard_map import shard_map

    run, sharded, in_names, out_avals = _get_runner()

    devices = jax.devices()[:NCORES]
    mesh = Mesh(np.asarray(devices), ("core",))
    sh = NamedSharding(mesh, PartitionSpec("core"))
    concat_in = [
        np.concatenate([np.asarray(in_maps[c][nm]) for c in range(NCORES)], axis=0)
        for nm in in_names
    ]
    dev_in = [jax.device_put(a, sh) for a in concat_in]
    # zeros are donated (consumed) per execution: pre-stage one set per iter
    znp = [np.zeros((NCORES * a.shape[0], *a.shape[1:]), a.dtype)
           for a in out_avals]
    zsets = [[jax.device_put(z, sh) for z in znp] for _ in range(iters + 1)]

    jax.block_until_ready(sharded(*dev_in, *zsets[-1]))  # warm

    if pipeline:
        t0 = time.perf_counter()
        outs = [sharded(*dev_in, *zsets[i]) for i in range(iters)]
        jax.block_until_ready(outs)
        tot = time.perf_counter() - t0
        return [tot / iters] * iters
    ts = []
    for i in range(iters):
        t0 = time.perf_counter()
        jax.block_until_ready(sharded(*dev_in, *zsets[i]))
        ts.append(time.perf_counter() - t0)
    return ts


if __name__ == "__main__":
    rng = np.random.default_rng(0)
    ins = {
        "hidden_states": rng.standard_normal((B, S, E), np.float32),
        "Wq": rng.standard_normal((E, E), np.float32) * E ** -0.5,
        "bq": rng.standard_normal(E).astype(np.float32) * 0.02,
        "Wk": rng.standard_normal((E, E), np.float32) * E ** -0.5,
        "bk": rng.standard_normal(E).astype(np.float32) * 0.02,
        "Wv": rng.standard_normal((E, E), np.float32) * E ** -0.5,
        "bv": rng.standard_normal(E).astype(np.float32) * 0.02,
        "Wo": rng.standard_normal((E, E), np.float32) * E ** -0.5,
        "bo": rng.standard_normal(E).astype(np.float32) * 0.02,
    }
    out = kernel(**ins)
    print(out.shape, out.dtype, np.abs(out).max())



# revision 24
# speedup vs baseline: 1.2504x; 1.2504x over previous
"""CLIP attention (B=4, S=2048, E=1024, H=16, D=64) on 8 Trainium2 cores.

Sharding: core c handles batch b = c // 2 and heads [ (c%2)*8, (c%2)*8+8 ).
Each core computes its 8 heads' attention plus its partial output
projection (contraction over its 512 local context dims); the host sums
the two partials per batch and adds the output bias.

Per-core dataflow (all activations stored transposed, [feature, seq]):
  hT [E, S]            <- host-pretransposed hidden_states[b], bf16
  qT, kT [512, S]      =  Wq_loc @ hT (+bias, query pre-scaled)   on PE
  v    [S, 512]        =  hT.T @ Wv_loc.T (+bias via bcast add), stored
                          as v_ext tiles [128, 8*65] with a ones column
                          per head (fused softmax denominator)
  ST   [k, q]          =  kT_h.T-slices @ qT_h  (scores, transposed;
                          two heads packed in PE row groups 0-63/64-127)
  P^T  = exp(ST)       on ACT, PSUM -> SBUF bf16 (no max subtraction:
                          scores ~ N(0,1), exp is safe in fp32)
  outT_ext [65, q]     =  V_ext.T @ P^T accumulated over k tiles; row 64
                          is the softmax denominator (ones column)
  CT   [512, S]        =  outT * (1/denom) broadcast  (DVE reciprocal,
                          PE ones-matmul partition broadcast, DVE mul)
  outT_partial [E, S]  =  Wo_loc^T-slices @ CT  -> DRAM fp32

Scheduling (measured on HW: PE matmul work ~300us/core, ACT exp ~246us,
total bounded by the PE stream + per-instruction sync overhead):
  - PSUM banks: 4 = two [128,1024] score tiles (score/exp pipeline),
    2 = per-head [65,512] AV accumulators, 2 = projection-chain /
    out-projection / 1/denom-broadcast tiles.
  - AV matmuls run 3 k-tiles behind scores/exp, so the sweep-boundary
    normalize (which frees the previous sweep's av slots on DVE) never
    stalls the PE.
  - q/k projection chains for head-pair p+1 are paced 2 per sweep
    (deadline-ordered: q0,k0,k1..k3,q1..q3) into pair p's sweeps;
    finished s-chunks' output projections fill the remaining slack.
"""

import numpy as np

B, S, E = 4, 2048, 1024
H, D = 16, 64
SCALE = D ** -0.5
NCORES = 8
HLOC = 8            # heads per core
CLOC = HLOC * D     # 512 local context dims
NHP = HLOC // 2     # 4 head pairs
SC = 512            # seq chunk (matmul moving free dim)
NQC = S // SC       # 4
KT = 128            # k tile rows
NKT = S // KT       # 16
NE = E // 128       # 8 contraction chunks for projections
VW = D + 1          # 65: v columns + fused ones column

_CACHE = {}


def _get_deps():
    import sys
    if "/opt/trn_rl_repo" not in sys.path:
        sys.path.insert(0, "/opt/trn_rl_repo")
    import concourse.bass as bass
    import concourse.mybir as mybir
    import concourse.tile as tile
    return bass, mybir, tile


def _fix_multi_waits(nc, mybir):
    """walrus encodes at most ONE semaphore wait per TPB engine
    instruction. Move surplus waits onto a same-engine Drain inserted just
    before the offending instruction (Drains accept many waits)."""
    for f in nc.m.functions:
        for bb in f.blocks:
            ins = bb.instructions
            if not any(i.sync_info and len(i.sync_info.on_wait) > 1
                       for i in ins):
                continue
            out = []
            for i in ins:
                if i.sync_info and len(i.sync_info.on_wait) > 1:
                    w = list(i.sync_info.on_wait)
                    # a wait on the instruction's OWN processor semaphore
                    # is implied by that processor's FIFO order - drop it
                    own = {u.ant_name for u in i.sync_info.on_update}
                    w2 = [x for x in w if x.ant_name not in own]
                    if not w2:
                        w2 = w[-1:]
                    for j, wj in enumerate(w2[:-1]):
                        d = mybir.InstDrain(
                            name=f"{i.name}_wj{j}", ins=[], outs=[],
                            bass_is_fusable=False)
                        d.engine = i.engine
                        d.sync_info = mybir.SyncInfo(on_wait=[wj], on_update=[])
                        out.append(d)
                    i.sync_info = mybir.SyncInfo(
                        on_wait=w2[-1:], on_update=list(i.sync_info.on_update))
                out.append(i)
            bb.instructions = out


def build_program(fix_waits=True, reps=1, probe=None):
    """Build the single-core Bass/Tile program (same program on all cores).

    fix_waits: apply the walrus 1-wait-per-instruction fixup (required for
    hardware NEFF compile; CoreSim's race detector can't digest the
    inserted drains, so sim runs pass False).
    probe: None for the real kernel; "noact" replaces the softmax exp with
    a constant tile (AV reads it) to time the pure PE stream."""
    bass, mybir, tile = _get_deps()
    from contextlib import ExitStack

    f32 = mybir.dt.float32
    bf16 = mybir.dt.bfloat16
    EXP = mybir.ActivationFunctionType.Exp

    nc = bass.Bass()

    hT_d = nc.declare_dram_parameter("hT", [E, S], bf16, isOutput=False)
    wqT_d = nc.declare_dram_parameter("wqT", [E, CLOC], bf16, isOutput=False)
    wkT_d = nc.declare_dram_parameter("wkT", [E, CLOC], bf16, isOutput=False)
    wvT_d = nc.declare_dram_parameter("wvT", [E, CLOC], bf16, isOutput=False)
    woT_d = nc.declare_dram_parameter("woT", [CLOC, E], bf16, isOutput=False)
    bq_d = nc.declare_dram_parameter("bq", [CLOC], f32, isOutput=False)
    bk_d = nc.declare_dram_parameter("bk", [CLOC], f32, isOutput=False)
    bv_d = nc.declare_dram_parameter("bv", [CLOC], f32, isOutput=False)
    outT_d = nc.declare_dram_parameter("outT", [E, S], f32, isOutput=True)

    add = mybir.AluOpType.add
    mult = mybir.AluOpType.mult

    with tile.TileContext(nc) as tc, ExitStack() as ctx:
        sb = ctx.enter_context(tc.tile_pool(name="persist", bufs=1))

        # ---- persistent SBUF tiles ----
        h_sb = [sb.tile([128, S], bf16, name=f"h{e}", tag=f"h{e}") for e in range(NE)]
        wq_sb = [sb.tile([128, CLOC], bf16, name=f"wq{e}", tag=f"wq{e}") for e in range(NE)]
        wk_sb = [sb.tile([128, CLOC], bf16, name=f"wk{e}", tag=f"wk{e}") for e in range(NE)]
        wv_sb = [sb.tile([128, CLOC], bf16, name=f"wv{e}", tag=f"wv{e}") for e in range(NE)]
        wo_sb = [sb.tile([128, E], bf16, name=f"wo{c}", tag=f"wo{c}") for c in range(4)]
        qT_sb = [sb.tile([128, S], bf16, name=f"qT{p}", tag=f"qT{p}") for p in range(NHP)]
        kT_sb = [sb.tile([128, S], bf16, name=f"kT{p}", tag=f"kT{p}") for p in range(NHP)]
        vx_sb = [sb.tile([128, HLOC * VW], bf16, name=f"vx{t}", tag=f"vx{t}") for t in range(NKT)]
        ct_sb = [sb.tile([128, S], bf16, name=f"ct{p}", tag=f"ct{p}") for p in range(NHP)]
        bq_sb = sb.tile([128, 4], f32, name="bq_sb", tag="bq_sb")
        bk_sb = sb.tile([128, 4], f32, name="bk_sb", tag="bk_sb")
        bv_sb = sb.tile([1, CLOC], f32, name="bv_sb", tag="bv_sb")
        bvb_sb = sb.tile([128, CLOC], f32, name="bvb_sb", tag="bvb_sb")
        ones1 = sb.tile([1, 128], f32, name="ones1", tag="ones1")
        ones64 = sb.tile([1, 64], bf16, name="ones64", tag="ones64")

        # ---- input DMAs, ordered by first use, 128 KB chunks so the
        # 8 hardware DMA queues stream them in parallel ----
        nc.sync.dma_start(out=bv_sb[:], in_=bv_d[:])
        for dtile in range(4):
            r = slice(dtile * 128, (dtile + 1) * 128)
            nc.sync.dma_start(out=bq_sb[:, dtile:dtile + 1], in_=bq_d[r])
            nc.sync.dma_start(out=bk_sb[:, dtile:dtile + 1], in_=bk_d[r])
        for e in range(NE):
            r = slice(e * 128, (e + 1) * 128)
            nc.sync.dma_start(out=wv_sb[e][:], in_=wvT_d[r, :])
        scol0 = slice(0, SC)
        for e in range(NE):
            r = slice(e * 128, (e + 1) * 128)
            nc.sync.dma_start(out=h_sb[e][:, scol0], in_=hT_d[r, scol0])
        for e in range(NE):
            r = slice(e * 128, (e + 1) * 128)
            nc.sync.dma_start(out=wq_sb[e][:], in_=wqT_d[r, :])
        for e in range(NE):
            r = slice(e * 128, (e + 1) * 128)
            nc.sync.dma_start(out=wk_sb[e][:], in_=wkT_d[r, :])
        for sc in range(1, NQC):
            scol = slice(sc * SC, (sc + 1) * SC)
            for e in range(NE):
                r = slice(e * 128, (e + 1) * 128)
                nc.sync.dma_start(out=h_sb[e][:, scol], in_=hT_d[r, scol])
        for c in range(4):
            nc.sync.dma_start(out=wo_sb[c][:], in_=woT_d[c * 128:(c + 1) * 128, :])
        nc.vector.memset(ones1[:], 1.0)
        nc.vector.memset(ones64[:], 1.0)
        ones_ex = None
        if probe == "noact":
            ones_ex = sb.tile([128, 2 * SC], bf16, name="ones_ex",
                              tag="ones_ex")
            nc.vector.memset(ones_ex[:], 0.001)

        # ones columns of v_ext (softmax denominator fusion), set once
        for t in range(NKT):
            for h in range(HLOC):
                nc.vector.memset(vx_sb[t][:, h * VW + D:h * VW + D + 1], 1.0)

        for _rep in range(reps):
            with tc.tile_pool(name="ppj", bufs=2, space="PSUM") as ppj, \
                 tc.tile_pool(name="avp", bufs=2, space="PSUM") as avp, \
                 tc.tile_pool(name="stp", bufs=2, space="PSUM") as stp, \
                 tc.tile_pool(name="exs", bufs=18) as exs, \
                 tc.tile_pool(name="nrm", bufs=4) as nrm, \
                 tc.tile_pool(name="ost", bufs=4) as ost:
                # PSUM budget (8 banks of 2KB):
                #   stp "st"  [128,1024] f32 x2 = 4 banks (score tiles)
                #   avp "av"  [VW, 512] f32 x2  = 2 banks (per-head AV
                #             accumulators)
                #   ppj "pps" [128, 512] f32 x2 = 2 banks (chains,
                #             out-projection, 1/denom broadcast)
                # AV matmuls run 3 k-tiles behind the score/exp pipeline so
                # the sweep-boundary normalize (which frees the previous
                # sweep's av slots) never stalls them.

                def qk_chain(w_sb, b_sb, dst_sb, p, scnk):
                    dcol = slice(p * 128, (p + 1) * 128)
                    scol = slice(scnk * SC, (scnk + 1) * SC)
                    ps = ppj.tile([128, SC], f32, name="qkps", tag="pps")
                    for e in range(NE):
                        nc.tensor.matmul(
                            ps[:], w_sb[e][:, dcol], h_sb[e][:, scol],
                            start=(e == 0), stop=(e == NE - 1),
                            skip_group_check=True)
                    nc.vector.tensor_scalar(
                        dst_sb[p][:, scol], ps[:], b_sb[:, p:p + 1],
                        None, op0=add)

                def v_chain(st):
                    ps = ppj.tile([128, CLOC], f32, name="vps", tag="pps")
                    for e in range(NE):
                        nc.tensor.matmul(
                            ps[:], h_sb[e][:, st * 128:(st + 1) * 128], wv_sb[e][:],
                            start=(e == 0), stop=(e == NE - 1))
                    nc.vector.tensor_tensor(
                        vx_sb[st][:].rearrange("p (h w) -> p h w", w=VW)[:, :, 0:D],
                        ps[:].rearrange("p (h w) -> p h w", w=D),
                        bvb_sb[:].rearrange("p (h w) -> p h w", w=D),
                        op=add)

                def out_proj_tile(scnk, et):
                    scol = slice(scnk * SC, (scnk + 1) * SC)
                    erow = slice(et * 128, (et + 1) * 128)
                    ps = ppj.tile([128, SC], f32, name="ops", tag="pps")
                    for c in range(4):
                        nc.tensor.matmul(
                            ps[:], wo_sb[c][:, erow], ct_sb[c][:, scol],
                            start=(c == 0), stop=(c == 3))
                    ot = ost.tile([128, SC], f32, name="ot", tag="ot")
                    nc.vector.tensor_copy(ot[:], ps[:])
                    nc.sync.dma_start(out=outT_d[erow, scol], in_=ot[:])

                # head: v-bias broadcast, the minimum projections the
                # first sweep needs (v tiles 0-3, q/k chunk 0 of pair 0);
                # everything else is interleaved into the sweeps
                bb_ps = ppj.tile([128, CLOC], f32, name="bbps", tag="pps")
                nc.tensor.matmul(bb_ps[:], ones1[:], bv_sb[:], start=True,
                                 stop=True)
                nc.vector.tensor_copy(bvb_sb[:], bb_ps[:])
                for st in range(4):
                    v_chain(st)
                qk_chain(wq_sb, bq_sb, qT_sb, 0, 0)
                qk_chain(wk_sb, bk_sb, kT_sb, 0, 0)

                # attention sweeps with PE filler work (q/k chains for the
                # next pair, out-projection for finished s-chunks) spread
                # into each sweep's slack
                chainq = []
                opq = []
                pending_norm = []

                def normalize(av, p, hh, qc):
                    # av: per-head [VW, SC] accumulator; row 64 = denom
                    qcol = slice(qc * SC, (qc + 1) * SC)
                    rr = nrm.tile([1, SC], bf16, name="rr", tag="rr")
                    with nc.allow_low_precision("1/denom in bf16: <=2^-9 rel"):
                        nc.vector.reciprocal(rr[:], av[64:65, :])
                    bc = ppj.tile([64, SC], f32, name="bc", tag="pps")
                    nc.tensor.matmul(bc[:], ones64[:], rr[0:1, :],
                                     start=True, stop=True)
                    rb = nrm.tile([64, SC], f32, name="rb", tag="rb")
                    nc.vector.tensor_copy(rb[:], bc[:])
                    nc.vector.tensor_tensor(
                        ct_sb[p][hh * 64:(hh + 1) * 64, qcol], av[0:64, :],
                        rb[:], op=mult)
                    if p == NHP - 1 and hh == 1:
                        for et in range(NE):
                            opq.append(
                                (lambda s=qc, e=et: out_proj_tile(s, e)))

                # first sweep's chain work, ordered by first use inside
                # the (p0, qc0) k-tile loop: k-chunks land before their
                # score tiles, v tiles before their AV matmuls
                for s in range(1, NQC):
                    chainq.append(
                        (lambda s=s: qk_chain(wk_sb, bk_sb, kT_sb, 0, s)))
                for st in range(4, NKT):
                    chainq.append((lambda st=st: v_chain(st)))
                # reorder: k1 first, then v4.., k2 after v6, k3 after v9
                chainq = [chainq[0], chainq[3], chainq[4], chainq[5],
                          chainq[1], chainq[6], chainq[7], chainq[8],
                          chainq[2]] + chainq[9:]
                for s in range(1, NQC):
                    chainq.append(
                        (lambda s=s: qk_chain(wq_sb, bq_sb, qT_sb, 0, s)))
                for p in range(NHP):
                    if p < NHP - 1:
                        # full chains for pair p+1, pulled 2 per sweep
                        # during pair p's sweeps; deadline order: q0/k0
                        # before the pair's first sweep, k1-k3 before that
                        # sweep's k-tiles 4/8/12, q1-q3 before sweeps 1-3
                        specs = ([(wq_sb, bq_sb, qT_sb, 0),
                                  (wk_sb, bk_sb, kT_sb, 0)] +
                                 [(wk_sb, bk_sb, kT_sb, s)
                                  for s in range(1, NQC)] +
                                 [(wq_sb, bq_sb, qT_sb, s)
                                  for s in range(1, NQC)])
                        for (w, b, d, s) in specs:
                            chainq.append(
                                (lambda w=w, b=b, d=d, s=s, q=p + 1:
                                 qk_chain(w, b, d, q, s)))
                    for qc in range(NQC):
                        qcol = slice(qc * SC, (qc + 1) * SC)
                        av = [avp.tile([VW, SC], f32, name=f"av{hh}",
                                       tag="av") for hh in range(2)]
                        exh = {}
                        dense = (p == 0 and qc == 0)
                        # k-tiles processed in batches: same-shape
                        # matmuls issue back-to-back (PE tile-config
                        # switches measure ~140ns each), scores+exp in
                        # batches of 2 (PSUM-limited), AV in batches of 4
                        # k-tiles running 5-8 k-tiles behind, which also
                        # keeps the sweep-boundary normalize off the
                        # critical path
                        for ktb in range(0, NKT + 6, 2):
                            for kt in (ktb, ktb + 1):
                                if kt >= NKT:
                                    break
                                kcol = slice(kt * 128, (kt + 1) * 128)
                                st_t = stp.tile([128, 2 * SC], f32, name="st",
                                                tag="st")
                                exh[kt] = st_t
                                nc.tensor.matmul(
                                    st_t[:, 0:SC], kT_sb[p][0:64, kcol],
                                    qT_sb[p][0:64, qcol],
                                    start=True, stop=True, tile_position=(0, 0))
                                nc.tensor.matmul(
                                    st_t[:, SC:2 * SC], kT_sb[p][64:128, kcol],
                                    qT_sb[p][64:128, qcol],
                                    start=True, stop=True, tile_position=(64, 0))
                            for kt in (ktb, ktb + 1):
                                if kt >= NKT:
                                    break
                                if probe == "noact":
                                    exh[kt] = ones_ex
                                else:
                                    ex = exs.tile([128, 2 * SC], bf16,
                                                  name="ex", tag="ex")
                                    nc.scalar.activation(ex[:], exh[kt][:],
                                                         EXP)
                                    exh[kt] = ex
                            # sweep-head normalize, emitted after the first
                            # score batch so the PE reaches the 1/denom
                            # broadcast matmuls only once the DVE
                            # reciprocals are done; frees the previous
                            # sweep's av slots before the AV matmuls need
                            # them
                            if ktb == 0:
                                while pending_norm:
                                    normalize(*pending_norm.pop(0))
                            if ktb in (12, 20):
                                for j in range(ktb - 12, ktb - 4):
                                    if j >= NKT:
                                        break
                                    pex = exh.pop(j)
                                    for hh in range(2):
                                        h = 2 * p + hh
                                        nc.tensor.matmul(
                                            av[hh][:],
                                            vx_sb[j][:, h * VW:h * VW + VW],
                                            pex[:, hh * SC:(hh + 1) * SC],
                                            start=(j == 0),
                                            stop=(j == NKT - 1),
                                            skip_group_check=True)
                            # filler work in the sweep's PE slack: the very
                            # first sweep pulls two chains per batch (it
                            # must produce the projections it consumes);
                            # later sweeps pace 2 chains per sweep and fill
                            # the rest with out-projection tiles
                            if dense and ktb < NKT and chainq:
                                chainq.pop(0)()
                                if chainq:
                                    chainq.pop(0)()
                            elif not dense and ktb in (2, 6) and chainq:
                                chainq.pop(0)()
                            elif not dense and ktb >= 4 and \
                                    ktb not in (2, 6) and opq:
                                opq.pop(0)()
                                if opq and ktb % 4 == 0:
                                    opq.pop(0)()
                        # normalize deferred into the next sweep's head
                        pending_norm.append((av[0], p, 0, qc))
                        pending_norm.append((av[1], p, 1, qc))
                while pending_norm:
                    normalize(*pending_norm.pop(0))
                # drain remaining out-projection tiles
                while opq:
                    opq.pop(0)()

    if fix_waits:
        _fix_multi_waits(nc, mybir)
    return nc


def make_inputs(hidden_states, Wq, bq, Wk, bk, Wv, bv, Wo, bo):
    """Shard + preprocess the full inputs into 8 per-core input maps.
    Shared pieces (per-batch hidden transpose, per-half weight slices)
    are computed once and referenced by both cores that use them."""
    import ml_dtypes
    bf16 = ml_dtypes.bfloat16
    f32 = np.float32

    hidden_states = np.asarray(hidden_states, f32)
    hT = [np.ascontiguousarray(hidden_states[b].T).astype(bf16)
          for b in range(B)]
    halves = []
    for half in range(2):
        hs = slice(half * CLOC, half * CLOC + CLOC)
        halves.append({
            "wqT": np.ascontiguousarray(
                (np.asarray(Wq, f32)[hs] * SCALE).T).astype(bf16),
            "wkT": np.ascontiguousarray(np.asarray(Wk, f32)[hs].T).astype(bf16),
            "wvT": np.ascontiguousarray(np.asarray(Wv, f32)[hs].T).astype(bf16),
            "woT": np.ascontiguousarray(np.asarray(Wo, f32)[:, hs].T).astype(bf16),
            "bq": np.ascontiguousarray(np.asarray(bq, f32)[hs] * SCALE),
            "bk": np.ascontiguousarray(np.asarray(bk, f32)[hs]),
            "bv": np.ascontiguousarray(np.asarray(bv, f32)[hs]),
        })
    return [{"hT": hT[c // 2], **halves[c % 2]} for c in range(NCORES)]


def gather_output(results, bo):
    out = np.empty((B, S, E), np.float32)
    bo = np.asarray(bo, np.float32)
    for b in range(B):
        acc = results[2 * b]["outT"].astype(np.float32) + \
              results[2 * b + 1]["outT"].astype(np.float32)
        out[b] = acc.T + bo
    return out


def _get_runner():
    """Build the Bass program + jitted 8-core executable once; reuse."""
    if "runner" in _CACHE:
        return _CACHE["runner"]
    _get_deps()
    import jax
    import numpy as np
    from jax.sharding import Mesh, PartitionSpec
    from jax.experimental.shard_map import shard_map
    from concourse import bass2jax, mybir

    bass2jax.install_neuronx_cc_hook()
    nc = build_program()

    partition_name = (nc.partition_id_tensor.name
                      if nc.partition_id_tensor else None)
    in_names, out_names, out_avals = [], [], []
    for alloc in nc.m.functions[0].allocations:
        if not isinstance(alloc, mybir.MemoryLocationSet):
            continue
        name = alloc.memorylocations[0].name
        if alloc.kind == "ExternalInput":
            if name != partition_name:
                in_names.append(name)
        elif alloc.kind == "ExternalOutput":
            out_names.append(name)
            out_avals.append(jax.core.ShapedArray(
                tuple(alloc.tensor_shape), mybir.dt.np(alloc.dtype)))
    n_params = len(in_names)
    all_in_names = in_names + out_names
    if partition_name is not None:
        all_in_names = all_in_names + [partition_name]

    def _body(*args):
        operands = list(args)
        if partition_name is not None:
            operands.append(bass2jax.partition_id_tensor())
        outs = bass2jax._bass_exec_p.bind(
            *operands,
            out_avals=tuple(out_avals),
            in_names=tuple(all_in_names),
            out_names=tuple(out_names),
            lowering_input_output_aliases=(),
            sim_require_finite=True,
            sim_require_nnan=True,
            nc=nc,
        )
        return tuple(outs)

    devices = jax.devices()[:NCORES]
    mesh = Mesh(np.asarray(devices), ("core",))
    n_outs = len(out_avals)
    sharded = jax.jit(
        shard_map(
            _body, mesh=mesh,
            in_specs=(PartitionSpec("core"),) * (n_params + n_outs),
            out_specs=(PartitionSpec("core"),) * n_outs,
            check_rep=False,
        ),
        donate_argnums=tuple(range(n_params, n_params + n_outs)),
        keep_unused=True,
    )

    def run(in_maps):
        concat_in = [
            np.concatenate([np.asarray(in_maps[c][nm]) for c in range(NCORES)],
                           axis=0)
            for nm in in_names
        ]
        concat_zeros = [
            np.zeros((NCORES * a.shape[0], *a.shape[1:]), a.dtype)
            for a in out_avals
        ]
        out_arrs = sharded(*concat_in, *concat_zeros)
        return [
            {nm: np.asarray(out_arrs[i]).reshape(NCORES, *out_avals[i].shape)[c]
             for i, nm in enumerate(out_names)}
            for c in range(NCORES)
        ]

    _CACHE["runner"] = (run, sharded, in_names, out_avals)
    return _CACHE["runner"]


def kernel(hidden_states, Wq, bq, Wk, bk, Wv, bv, Wo, bo):
    run = _get_runner()[0]
    in_maps = make_inputs(hidden_states, Wq, bq, Wk, bk, Wv, bv, Wo, bo)
    results = run(in_maps)
    return gather_output(results, bo)


def bench(in_maps, iters=20, pipeline=True):
    """Time repeated device executions with device-resident inputs and a
    non-donating jit (zeros reused). Returns per-iter seconds."""
    import time
    import numpy as np
    import jax
    from jax.sharding import Mesh, NamedSharding, PartitionSpec
    from jax.experimental.shard_map import shard_map

    run, sharded, in_names, out_avals = _get_runner()

    devices = jax.devices()[:NCORES]
    mesh = Mesh(np.asarray(devices), ("core",))
    sh = NamedSharding(mesh, PartitionSpec("core"))
    concat_in = [
        np.concatenate([np.asarray(in_maps[c][nm]) for c in range(NCORES)], axis=0)
        for nm in in_names
    ]
    dev_in = [jax.device_put(a, sh) for a in concat_in]
    # zeros are donated (consumed) per execution: pre-stage one set per iter
    znp = [np.zeros((NCORES * a.shape[0], *a.shape[1:]), a.dtype)
           for a in out_avals]
    zsets = [[jax.device_put(z, sh) for z in znp] for _ in range(iters + 1)]

    jax.block_until_ready(sharded(*dev_in, *zsets[-1]))  # warm

    if pipeline:
        t0 = time.perf_counter()
        outs = [sharded(*dev_in, *zsets[i]) for i in range(iters)]
        jax.block_until_ready(outs)
        tot = time.perf_counter() - t0
        return [tot / iters] * iters
    ts = []
    for i in range(iters):
        t0 = time.perf_counter()
        jax.block_until_ready(sharded(*dev_in, *zsets[i]))
        ts.append(time.perf_counter() - t0)
    return ts


if __name__ == "__main__":
    rng = np.random.default_rng(0)
    ins = {
        "hidden_states": rng.standard_normal((B, S, E), np.float32),
        "Wq": rng.standard_normal((E, E), np.float32) * E ** -0.5,
        "bq": rng.standard_normal(E).astype(np.float32) * 0.02,
        "Wk": rng.standard_normal((E, E), np.float32) * E ** -0.5,
        "bk": rng.standard_normal(E).astype(np.float32) * 0.02,
        "Wv": rng.standard_normal((E, E), np.float32) * E ** -0.5,
        "bv": rng.standard_normal(E).astype(np.float32) * 0.02,
        "Wo": rng.standard_normal((E, E), np.float32) * E ** -0.5,
        "bo": rng.standard_normal(E).astype(np.float32) * 0.02,
    }
    out = kernel(**ins)
    print(out.shape, out.dtype, np.abs(out).max())

